# revision 12
# baseline (speedup 1.0000x reference)
"""Trainium2 Bass kernel for nn_Adapter_XFormersAttnProcessor.

Data-parallel over batch: 16 batch elements -> 8 cores x 2 each. No
collectives. Each core computes, for its 2 batch elements:

    q = hs @ w_q.T; k/v = text @ w_k/v.T; bk/bv = box @ w_kb/vb.T;
    ck/cv = cls @ w_kc/vc.T
    bk2 = bk + mea(bk, ck, cv); bv2 = bv + mea(bv, ck, cv)
    out = (mea(q, k, v) + mea(q, bk2, bv2)) @ w_out.T + b_out

All matmuls run in bf16 (fp32 PSUM accumulation). Activations/weights are
pre-transposed on the host so every TensorEngine operand has its
contraction dim on the partition axis. Softmax is computed unnormalized
(exp then divide); the denominator is produced for free by augmenting the
value matrix with an all-ones block so that PSUM rows 64:128 of each
attention output hold the broadcast row-sums.
"""

import numpy as np
import ml_dtypes

BL = 2            # batch elements per core
NCORES = 8
B = 16
Q = 1024
C = 1280
CROSS = 2048
TT = 77           # text tokens
NB = 200          # box / cls tokens
H = 20            # heads
D = 64            # head dim
KC = C // 128     # 10 k-chunks over C
KX = CROSS // 128  # 16 k-chunks over CROSS
SCALE = float(np.float32(1.0) / np.sqrt(np.float32(D)))  # 0.125

BF16 = ml_dtypes.bfloat16

_CACHE = {}


def _build():
    if "nc" in _CACHE:
        return _CACHE["nc"]

    import concourse.tile as tile
    from concourse import bacc, mybir

    f32 = mybir.dt.float32
    bf16 = mybir.dt.bfloat16
    EXP = mybir.ActivationFunctionType.Exp
    IDENT = mybir.ActivationFunctionType.Identity
    ALU = mybir.AluOpType

    nc = bacc.Bacc(None, target_bir_lowering=False)

    hsT = nc.dram_tensor("hsT", [BL, C, Q], bf16, kind="ExternalInput")
    eT = nc.dram_tensor("eT", [BL, CROSS, TT + 2 * NB], bf16, kind="ExternalInput")
    wqT = nc.dram_tensor("wqT", [C, C], bf16, kind="ExternalInput")
    wkT = nc.dram_tensor("wkT", [CROSS, C], bf16, kind="ExternalInput")
    wvT = nc.dram_tensor("wvT", [CROSS, C], bf16, kind="ExternalInput")
    wkbT = nc.dram_tensor("wkbT", [CROSS, C], bf16, kind="ExternalInput")
    wvbT = nc.dram_tensor("wvbT", [CROSS, C], bf16, kind="ExternalInput")
    wkcT = nc.dram_tensor("wkcT", [CROSS, C], bf16, kind="ExternalInput")
    wvcT = nc.dram_tensor("wvcT", [CROSS, C], bf16, kind="ExternalInput")
    woutT = nc.dram_tensor("woutT", [C, C], bf16, kind="ExternalInput")
    b_out = nc.dram_tensor("b_out", [C, 1], f32, kind="ExternalInput")
    ident = nc.dram_tensor("ident", [128, 128], bf16, kind="ExternalInput")
    outT = nc.dram_tensor("outT", [BL, C, Q], f32, kind="ExternalOutput")

    with tile.TileContext(nc) as tc:
        with (
            tc.tile_pool(name="persist", bufs=1) as pp,
            tc.tile_pool(name="ps", bufs=1, space="PSUM") as ps,
        ):
            # ---------- persistent SBUF tensors ----------
            QT = [pp.tile([128, KC, Q], bf16, name=f"QT{b}") for b in range(BL)]
            # K.T for text, both batches side by side: cols b*TT+tok
            KT2 = pp.tile([128, KC, 2 * TT], bf16, name="KT2")
            # BK.T | BV.T per batch: cols b*400 + (0:200 bk | 200:400 bv)
            BKVT = pp.tile([128, KC, 4 * NB], bf16, name="BKVT")
            # CK.T: cols b*200+tok
            CKT = pp.tile([128, KC, 2 * NB], bf16, name="CKT")
            # V (text) token-major with per-head 64-col slots + shared ones block
            V1s = [pp.tile([128, 21 * D], bf16, name=f"V1s{b}") for b in range(BL)]
            # CV token-major (2 row-chunks of 128/72) + ones block
            CV1s = [pp.tile([128, 2, 21 * D], bf16, name=f"CV1s{b}") for b in range(BL)]
            # BV token-major (plain feature cols, no slots)
            BVtok = [pp.tile([128, 2, C], bf16, name=f"BVtok{b}") for b in range(BL)]
            # refined box keys, feature-major
            bk2T = [pp.tile([128, KC, NB], bf16, name=f"bk2T{b}") for b in range(BL)]
            # refined box values, token-major slots + ones block
            bv2 = [pp.tile([128, 2, 21 * D], bf16, name=f"bv2_{b}") for b in range(BL)]
            bias_sb = pp.tile([128, KC], f32, name="bias_sb")
            id_sb = pp.tile([128, 128], bf16, name="id_sb")

            nc.sync.dma_start(bias_sb[:], b_out.rearrange("(c p) o -> p (c o)", p=128))
            nc.sync.dma_start(id_sb[:], ident[:])

            V1s_r = [t.rearrange("p (s c) -> p s c", c=D) for t in V1s]
            CV1s_r = [t.rearrange("p k (s c) -> p k s c", c=D) for t in CV1s]
            bv2_r = [t.rearrange("p k (s c) -> p k s c", c=D) for t in bv2]
            for b in range(BL):
                nc.vector.memset(V1s_r[b][0:TT, 20, :], 1.0)
                nc.vector.memset(CV1s_r[b][:, :, 20, :], 1.0)
                nc.vector.memset(bv2_r[b][:, :, 20, :], 1.0)

            def blocks(t_r, h, *lead):
                """AP selecting per-head 64-col block h plus the ones block 20."""
                idx = lead + (slice(h, 21, 20 - h) if h < 20 else slice(20, 21),)
                return t_r[idx + (slice(0, D),)]

            # ---------- projections ----------
            with (
                tc.tile_pool(name="pw", bufs=1) as pw,
                tc.tile_pool(name="pet", bufs=1) as pet,
                tc.tile_pool(name="phs", bufs=2) as phs,
            ):
                ET = pet.tile([128, KX, BL, TT + 2 * NB], bf16, name="ET")
                for b in range(BL):
                    nc.sync.dma_start(
                        ET[:, :, b, :],
                        eT[b].rearrange("(c p) t -> p c t", p=128),
                    )

                # Q projection: QT[b] = (w_q @ hs[b].T), feature-major
                wq_sb = pw.tile([128, KC, C], bf16, name="w_sb", tag="w")
                nc.sync.dma_start(wq_sb[:], wqT.rearrange("(c p) o -> p c o", p=128))
                for b in range(BL):
                    for qc in range(2):
                        hs_t = phs.tile([128, KC, 512], bf16, name="hs_t")
                        nc.sync.dma_start(
                            hs_t[:],
                            hsT[b].rearrange("(c p) q -> p c q", p=128)[
                                :, :, qc * 512 : (qc + 1) * 512
                            ],
                        )
                        for co in range(KC):
                            acc = ps.tile([128, 512], f32, name="acc", tag="pa", bufs=3)
                            for k in range(KC):
                                nc.tensor.matmul(
                                    acc[:],
                                    wq_sb[:, k, co * 128 : (co + 1) * 128],
                                    hs_t[:, k, :],
                                    start=(k == 0),
                                    stop=(k == KC - 1),
                                )
                            nc.scalar.copy(
                                QT[b][:, co, qc * 512 : (qc + 1) * 512], acc[:]
                            )

                def load_w(dram):
                    w_sb = pw.tile([128, KX, C], bf16, name="w_sb", tag="w")
                    nc.sync.dma_start(
                        w_sb[:], dram.rearrange("(c p) o -> p c o", p=128)
                    )
                    return w_sb

                # text K (feature-major, both batches in free dim)
                w_sb = load_w(wkT)
                for co in range(KC):
                    acc = ps.tile([128, 2 * TT], f32, name="acc", tag="pa", bufs=3)
                    for k in range(KX):
                        nc.tensor.matmul(
                            acc[:],
                            w_sb[:, k, co * 128 : (co + 1) * 128],
                            ET[:, k, :, 0:TT],
                            start=(k == 0),
                            stop=(k == KX - 1),
                        )
                    nc.scalar.copy(KT2[:, co, :], acc[:])

                # text V (token-major into per-head slots)
                w_sb = load_w(wvT)
                for b in range(BL):
                    for no, nsz in ((0, 512), (1, 512), (2, 256)):
                        acc = ps.tile([128, 512], f32, name="acc", tag="pa", bufs=3)
                        for k in range(KX):
                            nc.tensor.matmul(
                                acc[0:TT, 0:nsz],
                                ET[:, k, b, 0:TT],
                                w_sb[:, k, no * 512 : no * 512 + nsz],
                                start=(k == 0),
                                stop=(k == KX - 1),
                            )
                        nc.scalar.copy(
                            V1s_r[b][0:TT, 8 * no : 8 * no + nsz // D, :],
                            acc[0:TT, 0:nsz],
                        )

                # box K / box V (feature-major)
                BKVT_r = BKVT.rearrange("p c (b s) -> p c b s", b=2)
                for wdram, soff in ((wkbT, 0), (wvbT, NB)):
                    w_sb = load_w(wdram)
                    for co in range(KC):
                        acc = ps.tile([128, 2 * NB], f32, name="acc", tag="pa", bufs=3)
                        for k in range(KX):
                            nc.tensor.matmul(
                                acc[:],
                                w_sb[:, k, co * 128 : (co + 1) * 128],
                                ET[:, k, :, TT : TT + NB],
                                start=(k == 0),
                                stop=(k == KX - 1),
                            )
                        nc.scalar.copy(
                            BKVT_r[:, co, :, soff : soff + NB], acc[:]
                        )

                # cls K (feature-major)
                w_sb = load_w(wkcT)
                for co in range(KC):
                    acc = ps.tile([128, 2 * NB], f32, name="acc", tag="pa", bufs=3)
                    for k in range(KX):
                        nc.tensor.matmul(
                            acc[:],
                            w_sb[:, k, co * 128 : (co + 1) * 128],
                            ET[:, k, :, TT + NB : TT + 2 * NB],
                            start=(k == 0),
                            stop=(k == KX - 1),
                        )
                    nc.scalar.copy(CKT[:, co, :], acc[:])

                # cls V (token-major slots, 2 row-chunks)
                w_sb = load_w(wvcT)
                for b in range(BL):
                    for tch, tsz in ((0, 128), (1, NB - 128)):
                        for no, nsz in ((0, 512), (1, 512), (2, 256)):
                            acc = ps.tile([128, 512], f32, name="acc", tag="pa", bufs=3)
                            t0 = TT + NB + tch * 128
                            for k in range(KX):
                                nc.tensor.matmul(
                                    acc[0:tsz, 0:nsz],
                                    ET[:, k, b, t0 : t0 + tsz],
                                    w_sb[:, k, no * 512 : no * 512 + nsz],
                                    start=(k == 0),
                                    stop=(k == KX - 1),
                                )
                            nc.scalar.copy(
                                CV1s_r[b][0:tsz, tch, 8 * no : 8 * no + nsz // D, :],
                                acc[0:tsz, 0:nsz],
                            )

                # BV token-major via PE transpose of BV.T
                for b in range(BL):
                    for co in range(KC):
                        for tch, tsz in ((0, 128), (1, NB - 128)):
                            tp = ps.tile([128, 128], bf16, name="tp", tag="pb", bufs=2)
                            nc.tensor.transpose(
                                tp[0:tsz, :],
                                BKVT_r[:, co, b, NB + tch * 128 : NB + tch * 128 + tsz],
                                id_sb[:],
                            )
                            nc.scalar.copy(
                                BVtok[b][0:tsz, tch, co * 128 : (co + 1) * 128],
                                tp[0:tsz, :],
                            )

            # ---------- attention ----------
            with (
                tc.tile_pool(name="pwo", bufs=1) as pwo,
                tc.tile_pool(name="patt", bufs=2) as pa,
                tc.tile_pool(name="pbig", bufs=2) as pbig,
            ):
                wo_sb = pwo.tile([128, KC, C], bf16, name="wo_sb")
                nc.sync.dma_start(wo_sb[:], woutT.rearrange("(c p) o -> p c o", p=128))

                for b in range(BL):
                    # ----- attn1: text attention, heads processed in pairs -----
                    O1 = [
                        pbig.tile([128, KC, 512], bf16, name="O1", tag="O1")
                        for _ in range(2)
                    ]
                    for qc in range(2):
                        for ch in range(KC):  # head pair (2*ch, 2*ch+1)
                            u1 = ps.tile([128, 512], f32, name="u1", tag="pb", bufs=2)
                            sb1 = ps.tile([128, 512], f32, name="sb1", tag="pc", bufs=2)
                            for hi in range(2):
                                h, rh = 2 * ch + hi, 64 * hi
                                s1 = ps.tile([128, 512], f32, name="s1", tag="pa", bufs=3)
                                nc.tensor.matmul(
                                    s1[0:TT, :],
                                    KT2[rh : rh + 64, ch, b * TT : (b + 1) * TT],
                                    QT[b][rh : rh + 64, ch, qc * 512 : (qc + 1) * 512],
                                    start=True,
                                    stop=True,
                                )
                                p1 = pa.tile([128, 512], bf16, name="p1", bufs=3)
                                nc.scalar.activation(
                                    p1[0:TT, :], s1[0:TT, :], EXP, scale=SCALE
                                )
                                nc.tensor.matmul(
                                    u1[rh : rh + 64, :],
                                    V1s_r[b][0:TT, h, :],
                                    p1[0:TT, :],
                                    start=True,
                                    stop=True,
                                    tile_position=(0, rh),
                                    skip_group_check=True,
                                )
                                nc.tensor.matmul(
                                    sb1[rh : rh + 64, :],
                                    V1s_r[b][0:TT, 20, :],
                                    p1[0:TT, :],
                                    start=True,
                                    stop=True,
                                    tile_position=(0, rh),
                                    skip_group_check=True,
                                )
                            rc1 = pa.tile([128, 512], f32, name="rc1", bufs=2)
                            nc.vector.reciprocal(rc1[:], sb1[:])
                            nc.vector.tensor_tensor(
                                O1[qc][:, ch, :], u1[:], rc1[:], ALU.mult
                            )

                    # ----- refine box K/V via cls attention (head pairs) -----
                    for ch in range(KC):
                        ur = ps.tile([128, NB], f32, name="ur", tag="pb", bufs=2)
                        sbr = ps.tile([128, NB], f32, name="sbr", tag="pc", bufs=2)
                        prs = []
                        for hi in range(2):
                            h, rh = 2 * ch + hi, 64 * hi
                            sra = ps.tile([128, 2 * NB], f32, name="sra", tag="pa", bufs=3)
                            srb = ps.tile([128, 2 * NB], f32, name="srb", tag="pa", bufs=3)
                            nc.tensor.matmul(
                                sra[:],
                                CKT[rh : rh + 64, ch, b * NB : b * NB + 128],
                                BKVT[rh : rh + 64, ch, b * 2 * NB : (b + 1) * 2 * NB],
                                start=True,
                                stop=True,
                            )
                            nc.tensor.matmul(
                                srb[0 : NB - 128, :],
                                CKT[rh : rh + 64, ch, b * NB + 128 : (b + 1) * NB],
                                BKVT[rh : rh + 64, ch, b * 2 * NB : (b + 1) * 2 * NB],
                                start=True,
                                stop=True,
                            )
                            pra = pa.tile([128, 2 * NB], bf16, name="pra", bufs=3)
                            prb = pa.tile([128, 2 * NB], bf16, name="prb", bufs=3)
                            nc.scalar.activation(pra[:], sra[:], EXP, scale=SCALE)
                            nc.scalar.activation(
                                prb[0 : NB - 128, :],
                                srb[0 : NB - 128, :],
                                EXP,
                                scale=SCALE,
                            )
                            prs.append((pra, prb))
                            # U for bk branch (feature-major), rows rh:rh+64
                            nc.tensor.matmul(
                                ur[rh : rh + 64, :],
                                CV1s_r[b][0:128, 0, h, :],
                                pra[:, 0:NB],
                                start=True,
                                stop=False,
                                tile_position=(0, rh),
                                skip_group_check=True,
                            )
                            nc.tensor.matmul(
                                ur[rh : rh + 64, :],
                                CV1s_r[b][0 : NB - 128, 1, h, :],
                                prb[0 : NB - 128, 0:NB],
                                start=False,
                                stop=True,
                                tile_position=(0, rh),
                                skip_group_check=True,
                            )
                            nc.tensor.matmul(
                                sbr[rh : rh + 64, :],
                                CV1s_r[b][0:128, 0, 20, :],
                                pra[:, 0:NB],
                                start=True,
                                stop=False,
                                tile_position=(0, rh),
                                skip_group_check=True,
                            )
                            nc.tensor.matmul(
                                sbr[rh : rh + 64, :],
                                CV1s_r[b][0 : NB - 128, 1, 20, :],
                                prb[0 : NB - 128, 0:NB],
                                start=False,
                                stop=True,
                                tile_position=(0, rh),
                                skip_group_check=True,
                            )
                        rcr = pa.tile([128, NB], f32, name="rcr", bufs=3)
                        nc.vector.reciprocal(rcr[:], sbr[:])
                        tmp = pa.tile([128, NB], f32, name="tmp", bufs=3)
                        nc.vector.tensor_tensor(tmp[:], ur[:], rcr[:], ALU.mult)
                        nc.vector.tensor_tensor(
                            bk2T[b][:, ch, :],
                            tmp[:],
                            BKVT[:, ch, b * 2 * NB : b * 2 * NB + NB],
                            ALU.add,
                        )
                        # token-major U for bv branch, per head
                        for hi in range(2):
                            h = 2 * ch + hi
                            pra, prb = prs[hi]
                            for tch, tsz in ((0, 128), (1, NB - 128)):
                                ubv = ps.tile(
                                    [128, 128], f32, name="ubv", tag="pc", bufs=2
                                )
                                nc.tensor.matmul(
                                    ubv[0:tsz, :],
                                    pra[:, NB + tch * 128 : NB + tch * 128 + tsz],
                                    blocks(CV1s_r[b], h, slice(0, 128), 0),
                                    start=True,
                                    stop=False,
                                )
                                nc.tensor.matmul(
                                    ubv[0:tsz, :],
                                    prb[
                                        0 : NB - 128,
                                        NB + tch * 128 : NB + tch * 128 + tsz,
                                    ],
                                    blocks(CV1s_r[b], h, slice(0, NB - 128), 1),
                                    start=False,
                                    stop=True,
                                )
                                rub = pa.tile([128, 1], f32, name="rub", bufs=2)
                                nc.vector.reciprocal(
                                    rub[0:tsz, :], ubv[0:tsz, 64:65]
                                )
                                nc.vector.scalar_tensor_tensor(
                                    bv2_r[b][0:tsz, tch, h, :],
                                    ubv[0:tsz, 0:64],
                                    rub[0:tsz, :],
                                    BVtok[b][0:tsz, tch, h * 64 : (h + 1) * 64],
                                    ALU.mult,
                                    ALU.add,
                                )

                    # ----- attn2 (box attention) + combine + out-projection -----
                    for qc in range(2):
                        OT = pbig.tile([128, KC, 512], bf16, name="OT", tag="OT", bufs=1)
                        for ch in range(KC):
                            u2 = ps.tile([128, 512], f32, name="u2", tag="pb", bufs=2)
                            sb2 = ps.tile([128, 512], f32, name="sb2", tag="pc", bufs=2)
                            for hi in range(2):
                                h, rh = 2 * ch + hi, 64 * hi
                                qs = QT[b][rh : rh + 64, ch, qc * 512 : (qc + 1) * 512]
                                s2a = ps.tile(
                                    [128, 512], f32, name="s2a", tag="pa", bufs=3
                                )
                                s2b = ps.tile(
                                    [128, 512], f32, name="s2b", tag="pa", bufs=3
                                )
                                nc.tensor.matmul(
                                    s2a[:],
                                    bk2T[b][rh : rh + 64, ch, 0:128],
                                    qs,
                                    start=True,
                                    stop=True,
                                )
                                nc.tensor.matmul(
                                    s2b[0 : NB - 128, :],
                                    bk2T[b][rh : rh + 64, ch, 128:NB],
                                    qs,
                                    start=True,
                                    stop=True,
                                )
                                p2a = pa.tile([128, 512], bf16, name="p2a", bufs=2)
                                p2b = pa.tile([128, 512], bf16, name="p2b", bufs=2)
                                nc.scalar.activation(p2a[:], s2a[:], EXP, scale=SCALE)
                                nc.scalar.activation(
                                    p2b[0 : NB - 128, :],
                                    s2b[0 : NB - 128, :],
                                    EXP,
                                    scale=SCALE,
                                )
                                nc.tensor.matmul(
                                    u2[rh : rh + 64, :],
                                    bv2_r[b][0:128, 0, h, :],
                                    p2a[:],
                                    start=True,
                                    stop=False,
                                    tile_position=(0, rh),
                                    skip_group_check=True,
                                )
                                nc.tensor.matmul(
                                    u2[rh : rh + 64, :],
                                    bv2_r[b][0 : NB - 128, 1, h, :],
                                    p2b[0 : NB - 128, :],
                                    start=False,
                                    stop=True,
                                    tile_position=(0, rh),
                                    skip_group_check=True,
                                )
                                nc.tensor.matmul(
                                    sb2[rh : rh + 64, :],
                                    bv2_r[b][0:128, 0, 20, :],
                                    p2a[:],
                                    start=True,
                                    stop=False,
                                    tile_position=(0, rh),
                                    skip_group_check=True,
                                )
                                nc.tensor.matmul(
                                    sb2[rh : rh + 64, :],
                                    bv2_r[b][0 : NB - 128, 1, 20, :],
                                    p2b[0 : NB - 128, :],
                                    start=False,
                                    stop=True,
                                    tile_position=(0, rh),
                                    skip_group_check=True,
                                )
                            rc2 = pa.tile([128, 512], f32, name="rc2", bufs=2)
                            nc.vector.reciprocal(rc2[:], sb2[:])
                            t2 = pa.tile([128, 512], bf16, name="t2", bufs=2)
                            nc.vector.tensor_tensor(t2[:], u2[:], rc2[:], ALU.mult)
                            nc.gpsimd.tensor_tensor(
                                OT[:, ch, :], t2[:], O1[qc][:, ch, :], ALU.add
                            )
                        # out-projection for this (b, qc)
                        for co in range(KC):
                            yp = ps.tile([128, 512], f32, name="yp", tag="pb", bufs=2)
                            for k in range(KC):
                                nc.tensor.matmul(
                                    yp[:],
                                    wo_sb[:, k, co * 128 : (co + 1) * 128],
                                    OT[:, k, :],
                                    start=(k == 0),
                                    stop=(k == KC - 1),
                                )
                            y = pa.tile([128, 512], f32, name="y", bufs=2)
                            nc.scalar.activation(
                                y[:], yp[:], IDENT, bias=bias_sb[:, co : co + 1]
                            )
                            nc.sync.dma_start(
                                outT[b][
                                    co * 128 : (co + 1) * 128,
                                    qc * 512 : (qc + 1) * 512,
                                ],
                                y[:],
                            )

    nc.compile()
    _CACHE["nc"] = nc
    return nc


def _prep_inputs(inputs):
    hs = np.asarray(inputs["hidden_states"], dtype=np.float32)
    ehs = np.asarray(inputs["encoder_hidden_states"], dtype=np.float32)
    hsT = np.ascontiguousarray(hs.transpose(0, 2, 1)).astype(BF16)
    eT = np.ascontiguousarray(ehs.transpose(0, 2, 1)).astype(BF16)

    def wt(name):
        return np.ascontiguousarray(
            np.asarray(inputs[name], dtype=np.float32).T
        ).astype(BF16)

    shared = {
        "wqT": wt("w_q"),
        "wkT": wt("w_k"),
        "wvT": wt("w_v"),
        "wkbT": wt("w_k_box"),
        "wvbT": wt("w_v_box"),
        "wkcT": wt("w_k_cls"),
        "wvcT": wt("w_v_cls"),
        "woutT": wt("w_out"),
        "b_out": np.ascontiguousarray(
            np.asarray(inputs["b_out"], dtype=np.float32).reshape(C, 1)
        ),
        "ident": np.eye(128, dtype=BF16),
    }
    in_maps = []
    for i in range(NCORES):
        m = dict(shared)
        m["hsT"] = np.ascontiguousarray(hsT[i * BL : (i + 1) * BL])
        m["eT"] = np.ascontiguousarray(eT[i * BL : (i + 1) * BL])
        in_maps.append(m)
    return in_maps


def _run(in_maps, trace=False):
    from concourse.bass_utils import run_bass_kernel_spmd

    nc = _build()
    return run_bass_kernel_spmd(
        nc, in_maps, core_ids=list(range(NCORES)), trace=trace
    )


def kernel(**inputs) -> np.ndarray:
    in_maps = _prep_inputs(inputs)
    res = _run(in_maps)
    outs = np.concatenate([r["outT"] for r in res.results], axis=0)  # [16, C, Q]
    return np.ascontiguousarray(outs.transpose(0, 2, 1)).astype(np.float32)


# revision 13
# speedup vs baseline: 1.1345x; 1.1345x over previous
"""Trainium2 Bass kernel for nn_Adapter_XFormersAttnProcessor.

Data-parallel over batch: 16 batch elements -> 8 cores x 2 each. No
collectives. Each core computes, for its 2 batch elements:

    q = hs @ w_q.T; k/v = text @ w_k/v.T; bk/bv = box @ w_kb/vb.T;
    ck/cv = cls @ w_kc/vc.T
    bk2 = bk + mea(bk, ck, cv); bv2 = bv + mea(bv, ck, cv)
    out = (mea(q, k, v) + mea(q, bk2, bv2)) @ w_out.T + b_out

All matmuls run in bf16 (fp32 PSUM accumulation). Activations/weights are
pre-transposed on the host so every TensorEngine operand has its
contraction dim on the partition axis. Softmax is computed unnormalized
(exp then divide); the denominator is produced for free by augmenting the
value matrix with an all-ones block so that PSUM rows 64:128 of each
attention output hold the broadcast row-sums.
"""

import numpy as np
import ml_dtypes

BL = 2            # batch elements per core
NCORES = 8
B = 16
Q = 1024
C = 1280
CROSS = 2048
TT = 77           # text tokens
NB = 200          # box / cls tokens
H = 20            # heads
D = 64            # head dim
KC = C // 128     # 10 k-chunks over C
KX = CROSS // 128  # 16 k-chunks over CROSS
SCALE = float(np.float32(1.0) / np.sqrt(np.float32(D)))  # 0.125

BF16 = ml_dtypes.bfloat16

_CACHE = {}


def _build():
    if "nc" in _CACHE:
        return _CACHE["nc"]

    import concourse.tile as tile
    from concourse import bacc, mybir

    f32 = mybir.dt.float32
    bf16 = mybir.dt.bfloat16
    EXP = mybir.ActivationFunctionType.Exp
    IDENT = mybir.ActivationFunctionType.Identity
    ALU = mybir.AluOpType

    nc = bacc.Bacc(None, target_bir_lowering=False)

    hsT = nc.dram_tensor("hsT", [BL, C, Q], bf16, kind="ExternalInput")
    eT = nc.dram_tensor("eT", [BL, CROSS, TT + 2 * NB], bf16, kind="ExternalInput")
    wqT = nc.dram_tensor("wqT", [C, C], bf16, kind="ExternalInput")
    wkT = nc.dram_tensor("wkT", [CROSS, C], bf16, kind="ExternalInput")
    wvT = nc.dram_tensor("wvT", [CROSS, C], bf16, kind="ExternalInput")
    wkbT = nc.dram_tensor("wkbT", [CROSS, C], bf16, kind="ExternalInput")
    wvbT = nc.dram_tensor("wvbT", [CROSS, C], bf16, kind="ExternalInput")
    wkcT = nc.dram_tensor("wkcT", [CROSS, C], bf16, kind="ExternalInput")
    wvcT = nc.dram_tensor("wvcT", [CROSS, C], bf16, kind="ExternalInput")
    woutT = nc.dram_tensor("woutT", [C, C], bf16, kind="ExternalInput")
    b_out = nc.dram_tensor("b_out", [C, 1], f32, kind="ExternalInput")
    ident = nc.dram_tensor("ident", [128, 128], bf16, kind="ExternalInput")
    outT = nc.dram_tensor("outT", [BL, C, Q], f32, kind="ExternalOutput")

    with tile.TileContext(nc) as tc:
        with (
            tc.tile_pool(name="persist", bufs=1) as pp,
            tc.tile_pool(name="ps", bufs=1, space="PSUM") as ps,
        ):
            # ---------- persistent SBUF tensors ----------
            QT = [pp.tile([128, KC, Q], bf16, name=f"QT{b}") for b in range(BL)]
            # K.T for text, both batches side by side: cols b*TT+tok
            KT2 = pp.tile([128, KC, 2 * TT], bf16, name="KT2")
            # BK.T | BV.T per batch: cols b*400 + (0:200 bk | 200:400 bv)
            BKVT = pp.tile([128, KC, 4 * NB], bf16, name="BKVT")
            # CK.T: cols b*200+tok
            CKT = pp.tile([128, KC, 2 * NB], bf16, name="CKT")
            # V (text) token-major with per-head 64-col slots + shared ones block
            V1s = [pp.tile([128, 21 * D], bf16, name=f"V1s{b}") for b in range(BL)]
            # CV token-major (2 row-chunks of 128/72) + ones block
            CV1s = [pp.tile([128, 2, 21 * D], bf16, name=f"CV1s{b}") for b in range(BL)]
            # BV token-major (plain feature cols, no slots)
            BVtok = [pp.tile([128, 2, C], bf16, name=f"BVtok{b}") for b in range(BL)]
            # refined box keys, feature-major
            bk2T = [pp.tile([128, KC, NB], bf16, name=f"bk2T{b}") for b in range(BL)]
            # refined box values, token-major slots + ones block
            bv2 = [pp.tile([128, 2, 21 * D], bf16, name=f"bv2_{b}") for b in range(BL)]
            bias_sb = pp.tile([128, KC], f32, name="bias_sb")
            id_sb = pp.tile([128, 128], bf16, name="id_sb")

            nc.sync.dma_start(bias_sb[:], b_out.rearrange("(c p) o -> p (c o)", p=128))
            nc.sync.dma_start(id_sb[:], ident[:])

            V1s_r = [t.rearrange("p (s c) -> p s c", c=D) for t in V1s]
            CV1s_r = [t.rearrange("p k (s c) -> p k s c", c=D) for t in CV1s]
            bv2_r = [t.rearrange("p k (s c) -> p k s c", c=D) for t in bv2]
            for b in range(BL):
                nc.vector.memset(V1s_r[b][0:TT, 20, :], 1.0)
                nc.vector.memset(CV1s_r[b][:, :, 20, :], 1.0)
                nc.vector.memset(bv2_r[b][:, :, 20, :], 1.0)

            def blocks(t_r, h, *lead):
                """AP selecting per-head 64-col block h plus the ones block 20."""
                idx = lead + (slice(h, 21, 20 - h) if h < 20 else slice(20, 21),)
                return t_r[idx + (slice(0, D),)]

            # ---------- projections ----------
            with (
                tc.tile_pool(name="pw", bufs=1) as pw,
                tc.tile_pool(name="pet", bufs=1) as pet,
                tc.tile_pool(name="phs", bufs=2) as phs,
            ):
                ET = pet.tile([128, KX, BL, TT + 2 * NB], bf16, name="ET")
                for b in range(BL):
                    nc.sync.dma_start(
                        ET[:, :, b, :],
                        eT[b].rearrange("(c p) t -> p c t", p=128),
                    )

                # Q projection: QT[b] = (w_q @ hs[b].T), feature-major
                wq_sb = pw.tile([128, KC, C], bf16, name="w_sb", tag="w")
                nc.sync.dma_start(wq_sb[:], wqT.rearrange("(c p) o -> p c o", p=128))
                for b in range(BL):
                    for qc in range(2):
                        hs_t = phs.tile([128, KC, 512], bf16, name="hs_t")
                        nc.sync.dma_start(
                            hs_t[:],
                            hsT[b].rearrange("(c p) q -> p c q", p=128)[
                                :, :, qc * 512 : (qc + 1) * 512
                            ],
                        )
                        for co in range(KC):
                            acc = ps.tile([128, 512], f32, name="acc", tag="pa", bufs=3)
                            for k in range(KC):
                                nc.tensor.matmul(
                                    acc[:],
                                    wq_sb[:, k, co * 128 : (co + 1) * 128],
                                    hs_t[:, k, :],
                                    start=(k == 0),
                                    stop=(k == KC - 1),
                                )
                            nc.scalar.copy(
                                QT[b][:, co, qc * 512 : (qc + 1) * 512], acc[:]
                            )

                def load_w(dram):
                    w_sb = pw.tile([128, KX, C], bf16, name="w_sb", tag="w")
                    nc.sync.dma_start(
                        w_sb[:], dram.rearrange("(c p) o -> p c o", p=128)
                    )
                    return w_sb

                # text K (feature-major, both batches in free dim)
                w_sb = load_w(wkT)
                for co in range(KC):
                    acc = ps.tile([128, 2 * TT], f32, name="acc", tag="pa", bufs=3)
                    for k in range(KX):
                        nc.tensor.matmul(
                            acc[:],
                            w_sb[:, k, co * 128 : (co + 1) * 128],
                            ET[:, k, :, 0:TT],
                            start=(k == 0),
                            stop=(k == KX - 1),
                        )
                    nc.scalar.copy(KT2[:, co, :], acc[:])

                # text V (token-major into per-head slots)
                w_sb = load_w(wvT)
                for b in range(BL):
                    for no, nsz in ((0, 512), (1, 512), (2, 256)):
                        acc = ps.tile([128, 512], f32, name="acc", tag="pa", bufs=3)
                        for k in range(KX):
                            nc.tensor.matmul(
                                acc[0:TT, 0:nsz],
                                ET[:, k, b, 0:TT],
                                w_sb[:, k, no * 512 : no * 512 + nsz],
                                start=(k == 0),
                                stop=(k == KX - 1),
                            )
                        nc.scalar.copy(
                            V1s_r[b][0:TT, 8 * no : 8 * no + nsz // D, :],
                            acc[0:TT, 0:nsz],
                        )

                # box K / box V (feature-major)
                BKVT_r = BKVT.rearrange("p c (b s) -> p c b s", b=2)
                for wdram, soff in ((wkbT, 0), (wvbT, NB)):
                    w_sb = load_w(wdram)
                    for co in range(KC):
                        acc = ps.tile([128, 2 * NB], f32, name="acc", tag="pa", bufs=3)
                        for k in range(KX):
                            nc.tensor.matmul(
                                acc[:],
                                w_sb[:, k, co * 128 : (co + 1) * 128],
                                ET[:, k, :, TT : TT + NB],
                                start=(k == 0),
                                stop=(k == KX - 1),
                            )
                        nc.scalar.copy(
                            BKVT_r[:, co, :, soff : soff + NB], acc[:]
                        )

                # cls K (feature-major)
                w_sb = load_w(wkcT)
                for co in range(KC):
                    acc = ps.tile([128, 2 * NB], f32, name="acc", tag="pa", bufs=3)
                    for k in range(KX):
                        nc.tensor.matmul(
                            acc[:],
                            w_sb[:, k, co * 128 : (co + 1) * 128],
                            ET[:, k, :, TT + NB : TT + 2 * NB],
                            start=(k == 0),
                            stop=(k == KX - 1),
                        )
                    nc.scalar.copy(CKT[:, co, :], acc[:])

                # cls V (token-major slots, 2 row-chunks)
                w_sb = load_w(wvcT)
                for b in range(BL):
                    for tch, tsz in ((0, 128), (1, NB - 128)):
                        for no, nsz in ((0, 512), (1, 512), (2, 256)):
                            acc = ps.tile([128, 512], f32, name="acc", tag="pa", bufs=3)
                            t0 = TT + NB + tch * 128
                            for k in range(KX):
                                nc.tensor.matmul(
                                    acc[0:tsz, 0:nsz],
                                    ET[:, k, b, t0 : t0 + tsz],
                                    w_sb[:, k, no * 512 : no * 512 + nsz],
                                    start=(k == 0),
                                    stop=(k == KX - 1),
                                )
                            nc.scalar.copy(
                                CV1s_r[b][0:tsz, tch, 8 * no : 8 * no + nsz // D, :],
                                acc[0:tsz, 0:nsz],
                            )

                # BV token-major via PE transpose of BV.T
                for b in range(BL):
                    for co in range(KC):
                        for tch, tsz in ((0, 128), (1, NB - 128)):
                            tp = ps.tile([128, 128], bf16, name="tp", tag="pb", bufs=2)
                            nc.tensor.transpose(
                                tp[0:tsz, :],
                                BKVT_r[:, co, b, NB + tch * 128 : NB + tch * 128 + tsz],
                                id_sb[:],
                            )
                            nc.scalar.copy(
                                BVtok[b][0:tsz, tch, co * 128 : (co + 1) * 128],
                                tp[0:tsz, :],
                            )

            # ---------- attention ----------
            with (
                tc.tile_pool(name="pwo", bufs=1) as pwo,
                tc.tile_pool(name="patt", bufs=2) as pa,
                tc.tile_pool(name="pbig", bufs=2) as pbig,
            ):
                wo_sb = pwo.tile([128, KC, C], bf16, name="wo_sb")
                nc.sync.dma_start(wo_sb[:], woutT.rearrange("(c p) o -> p c o", p=128))

                for b in range(BL):
                    # ----- attn1: text attention, heads processed in pairs -----
                    O1 = [
                        pbig.tile([128, KC, 512], bf16, name="O1", tag="O1")
                        for _ in range(2)
                    ]
                    for qc in range(2):
                        for ch in range(KC):  # head pair (2*ch, 2*ch+1)
                            u1 = ps.tile([128, 512], f32, name="u1", tag="pb", bufs=2)
                            sb1 = ps.tile([128, 512], f32, name="sb1", tag="pc", bufs=2)
                            for hi in range(2):
                                h, rh = 2 * ch + hi, 64 * hi
                                s1 = ps.tile([128, 512], f32, name="s1", tag="pa", bufs=3)
                                nc.tensor.matmul(
                                    s1[0:TT, :],
                                    KT2[rh : rh + 64, ch, b * TT : (b + 1) * TT],
                                    QT[b][rh : rh + 64, ch, qc * 512 : (qc + 1) * 512],
                                    start=True,
                                    stop=True,
                                )
                                p1 = pa.tile([128, 512], bf16, name="p1", bufs=3)
                                nc.scalar.activation(
                                    p1[0:TT, :], s1[0:TT, :], EXP, scale=SCALE
                                )
                                nc.tensor.matmul(
                                    u1[rh : rh + 64, :],
                                    V1s_r[b][0:TT, h, :],
                                    p1[0:TT, :],
                                    start=True,
                                    stop=True,
                                    tile_position=(0, rh),
                                    skip_group_check=True,
                                )
                                nc.tensor.matmul(
                                    sb1[rh : rh + 64, :],
                                    V1s_r[b][0:TT, 20, :],
                                    p1[0:TT, :],
                                    start=True,
                                    stop=True,
                                    tile_position=(0, rh),
                                    skip_group_check=True,
                                )
                            rc1 = pa.tile([128, 512], f32, name="rc1", bufs=2)
                            nc.vector.reciprocal_approx_fast(rc1[:], sb1[:])
                            nc.vector.tensor_tensor(
                                O1[qc][:, ch, :], u1[:], rc1[:], ALU.mult
                            )

                    # ----- refine box K/V via cls attention (head pairs) -----
                    for ch in range(KC):
                        ur = ps.tile([128, NB], f32, name="ur", tag="pb", bufs=2)
                        sbr = ps.tile([128, NB], f32, name="sbr", tag="pc", bufs=2)
                        prs = []
                        for hi in range(2):
                            h, rh = 2 * ch + hi, 64 * hi
                            sra = ps.tile([128, 2 * NB], f32, name="sra", tag="pa", bufs=3)
                            srb = ps.tile([128, 2 * NB], f32, name="srb", tag="pa", bufs=3)
                            nc.tensor.matmul(
                                sra[:],
                                CKT[rh : rh + 64, ch, b * NB : b * NB + 128],
                                BKVT[rh : rh + 64, ch, b * 2 * NB : (b + 1) * 2 * NB],
                                start=True,
                                stop=True,
                            )
                            nc.tensor.matmul(
                                srb[0 : NB - 128, :],
                                CKT[rh : rh + 64, ch, b * NB + 128 : (b + 1) * NB],
                                BKVT[rh : rh + 64, ch, b * 2 * NB : (b + 1) * 2 * NB],
                                start=True,
                                stop=True,
                            )
                            pra = pa.tile([128, 2 * NB], bf16, name="pra", bufs=3)
                            prb = pa.tile([128, 2 * NB], bf16, name="prb", bufs=3)
                            nc.scalar.activation(pra[:], sra[:], EXP, scale=SCALE)
                            nc.scalar.activation(
                                prb[0 : NB - 128, :],
                                srb[0 : NB - 128, :],
                                EXP,
                                scale=SCALE,
                            )
                            prs.append((pra, prb))
                            # U for bk branch (feature-major), rows rh:rh+64
                            nc.tensor.matmul(
                                ur[rh : rh + 64, :],
                                CV1s_r[b][0:128, 0, h, :],
                                pra[:, 0:NB],
                                start=True,
                                stop=False,
                                tile_position=(0, rh),
                                skip_group_check=True,
                            )
                            nc.tensor.matmul(
                                ur[rh : rh + 64, :],
                                CV1s_r[b][0 : NB - 128, 1, h, :],
                                prb[0 : NB - 128, 0:NB],
                                start=False,
                                stop=True,
                                tile_position=(0, rh),
                                skip_group_check=True,
                            )
                            nc.tensor.matmul(
                                sbr[rh : rh + 64, :],
                                CV1s_r[b][0:128, 0, 20, :],
                                pra[:, 0:NB],
                                start=True,
                                stop=False,
                                tile_position=(0, rh),
                                skip_group_check=True,
                            )
                            nc.tensor.matmul(
                                sbr[rh : rh + 64, :],
                                CV1s_r[b][0 : NB - 128, 1, 20, :],
                                prb[0 : NB - 128, 0:NB],
                                start=False,
                                stop=True,
                                tile_position=(0, rh),
                                skip_group_check=True,
                            )
                        rcr = pa.tile([128, NB], f32, name="rcr", bufs=3)
                        nc.vector.reciprocal_approx_fast(rcr[:], sbr[:])
                        tmp = pa.tile([128, NB], f32, name="tmp", bufs=3)
                        nc.vector.tensor_tensor(tmp[:], ur[:], rcr[:], ALU.mult)
                        nc.vector.tensor_tensor(
                            bk2T[b][:, ch, :],
                            tmp[:],
                            BKVT[:, ch, b * 2 * NB : b * 2 * NB + NB],
                            ALU.add,
                        )
                        # token-major U for bv branch, per head
                        for hi in range(2):
                            h = 2 * ch + hi
                            pra, prb = prs[hi]
                            for tch, tsz in ((0, 128), (1, NB - 128)):
                                ubv = ps.tile(
                                    [128, 128], f32, name="ubv", tag="pc", bufs=2
                                )
                                nc.tensor.matmul(
                                    ubv[0:tsz, :],
                                    pra[:, NB + tch * 128 : NB + tch * 128 + tsz],
                                    blocks(CV1s_r[b], h, slice(0, 128), 0),
                                    start=True,
                                    stop=False,
                                )
                                nc.tensor.matmul(
                                    ubv[0:tsz, :],
                                    prb[
                                        0 : NB - 128,
                                        NB + tch * 128 : NB + tch * 128 + tsz,
                                    ],
                                    blocks(CV1s_r[b], h, slice(0, NB - 128), 1),
                                    start=False,
                                    stop=True,
                                )
                                rub = pa.tile([128, 1], f32, name="rub", bufs=2)
                                nc.vector.reciprocal_approx_fast(
                                    rub[0:tsz, :], ubv[0:tsz, 64:65]
                                )
                                nc.vector.scalar_tensor_tensor(
                                    bv2_r[b][0:tsz, tch, h, :],
                                    ubv[0:tsz, 0:64],
                                    rub[0:tsz, :],
                                    BVtok[b][0:tsz, tch, h * 64 : (h + 1) * 64],
                                    ALU.mult,
                                    ALU.add,
                                )

                    # ----- attn2 (box attention) + combine + out-projection -----
                    for qc in range(2):
                        OT = pbig.tile([128, KC, 512], bf16, name="OT", tag="OT", bufs=1)
                        for ch in range(KC):
                            u2 = ps.tile([128, 512], f32, name="u2", tag="pb", bufs=2)
                            sb2 = ps.tile([128, 512], f32, name="sb2", tag="pc", bufs=2)
                            for hi in range(2):
                                h, rh = 2 * ch + hi, 64 * hi
                                qs = QT[b][rh : rh + 64, ch, qc * 512 : (qc + 1) * 512]
                                s2a = ps.tile(
                                    [128, 512], f32, name="s2a", tag="pa", bufs=3
                                )
                                s2b = ps.tile(
                                    [128, 512], f32, name="s2b", tag="pa", bufs=3
                                )
                                nc.tensor.matmul(
                                    s2a[:],
                                    bk2T[b][rh : rh + 64, ch, 0:128],
                                    qs,
                                    start=True,
                                    stop=True,
                                )
                                nc.tensor.matmul(
                                    s2b[0 : NB - 128, :],
                                    bk2T[b][rh : rh + 64, ch, 128:NB],
                                    qs,
                                    start=True,
                                    stop=True,
                                )
                                p2a = pa.tile([128, 512], bf16, name="p2a", bufs=2)
                                p2b = pa.tile([128, 512], bf16, name="p2b", bufs=2)
                                nc.scalar.activation(p2a[:], s2a[:], EXP, scale=SCALE)
                                nc.scalar.activation(
                                    p2b[0 : NB - 128, :],
                                    s2b[0 : NB - 128, :],
                                    EXP,
                                    scale=SCALE,
                                )
                                nc.tensor.matmul(
                                    u2[rh : rh + 64, :],
                                    bv2_r[b][0:128, 0, h, :],
                                    p2a[:],
                                    start=True,
                                    stop=False,
                                    tile_position=(0, rh),
                                    skip_group_check=True,
                                )
                                nc.tensor.matmul(
                                    u2[rh : rh + 64, :],
                                    bv2_r[b][0 : NB - 128, 1, h, :],
                                    p2b[0 : NB - 128, :],
                                    start=False,
                                    stop=True,
                                    tile_position=(0, rh),
                                    skip_group_check=True,
                                )
                                nc.tensor.matmul(
                                    sb2[rh : rh + 64, :],
                                    bv2_r[b][0:128, 0, 20, :],
                                    p2a[:],
                                    start=True,
                                    stop=False,
                                    tile_position=(0, rh),
                                    skip_group_check=True,
                                )
                                nc.tensor.matmul(
                                    sb2[rh : rh + 64, :],
                                    bv2_r[b][0 : NB - 128, 1, 20, :],
                                    p2b[0 : NB - 128, :],
                                    start=False,
                                    stop=True,
                                    tile_position=(0, rh),
                                    skip_group_check=True,
                                )
                            rc2 = pa.tile([128, 512], f32, name="rc2", bufs=2)
                            nc.vector.reciprocal_approx_fast(rc2[:], sb2[:])
                            t2 = pa.tile([128, 512], bf16, name="t2", bufs=2)
                            nc.vector.tensor_tensor(t2[:], u2[:], rc2[:], ALU.mult)
                            nc.gpsimd.tensor_tensor(
                                OT[:, ch, :], t2[:], O1[qc][:, ch, :], ALU.add
                            )
                        # out-projection for this (b, qc)
                        for co in range(KC):
                            yp = ps.tile([128, 512], f32, name="yp", tag="pb", bufs=2)
                            for k in range(KC):
                                nc.tensor.matmul(
                                    yp[:],
                                    wo_sb[:, k, co * 128 : (co + 1) * 128],
                                    OT[:, k, :],
                                    start=(k == 0),
                                    stop=(k == KC - 1),
                                )
                            y = pa.tile([128, 512], f32, name="y", bufs=2)
                            nc.scalar.activation(
                                y[:], yp[:], IDENT, bias=bias_sb[:, co : co + 1]
                            )
                            nc.sync.dma_start(
                                outT[b][
                                    co * 128 : (co + 1) * 128,
                                    qc * 512 : (qc + 1) * 512,
                                ],
                                y[:],
                            )

    nc.compile()
    _CACHE["nc"] = nc
    return nc


def _prep_inputs(inputs):
    hs = np.asarray(inputs["hidden_states"], dtype=np.float32)
    ehs = np.asarray(inputs["encoder_hidden_states"], dtype=np.float32)
    hsT = np.ascontiguousarray(hs.transpose(0, 2, 1)).astype(BF16)
    eT = np.ascontiguousarray(ehs.transpose(0, 2, 1)).astype(BF16)

    def wt(name):
        return np.ascontiguousarray(
            np.asarray(inputs[name], dtype=np.float32).T
        ).astype(BF16)

    shared = {
        "wqT": wt("w_q"),
        "wkT": wt("w_k"),
        "wvT": wt("w_v"),
        "wkbT": wt("w_k_box"),
        "wvbT": wt("w_v_box"),
        "wkcT": wt("w_k_cls"),
        "wvcT": wt("w_v_cls"),
        "woutT": wt("w_out"),
        "b_out": np.ascontiguousarray(
            np.asarray(inputs["b_out"], dtype=np.float32).reshape(C, 1)
        ),
        "ident": np.eye(128, dtype=BF16),
    }
    in_maps = []
    for i in range(NCORES):
        m = dict(shared)
        m["hsT"] = np.ascontiguousarray(hsT[i * BL : (i + 1) * BL])
        m["eT"] = np.ascontiguousarray(eT[i * BL : (i + 1) * BL])
        in_maps.append(m)
    return in_maps


def _run(in_maps, trace=False):
    from concourse.bass_utils import run_bass_kernel_spmd

    nc = _build()
    return run_bass_kernel_spmd(
        nc, in_maps, core_ids=list(range(NCORES)), trace=trace
    )


def kernel(**inputs) -> np.ndarray:
    in_maps = _prep_inputs(inputs)
    res = _run(in_maps)
    outs = np.concatenate([r["outT"] for r in res.results], axis=0)  # [16, C, Q]
    return np.ascontiguousarray(outs.transpose(0, 2, 1)).astype(np.float32)
